# revision 1
# baseline (speedup 1.0000x reference)
"""BitLinear (ternary-quantized linear) Trainium2 kernel, 8-way tensor-parallel.

Computes  out = x @ quantize(weight).T + bias  for
  x      (8192, 4096) f32
  weight (16384, 4096) f32
  bias   (16384,) f32
  out    (8192, 16384) f32

quantize(w) = ternarize(w / scale) * scale with scale = max(mean|w|, 1e-6),
ternary in {-1, 0, +1}.

Strategy (column-parallel linear per the tensor-parallel sharding):
  - Host: compute scale, ternarize weights (exactly representable in fp16),
    cast x to fp16, pre-transpose both so the device does no transposes.
  - Each of the 8 cores holds a 2048-wide slice of out_features, streams the
    full x once, and computes outT_c = (wT_c.T @ xT) with fp32 PSUM
    accumulation; the ACT engine applies  *scale + bias  on PSUM eviction.
  - No collectives: the host concatenates the 8 column slices.

Device layout per core (out^T orientation — out_features on partitions):
  lhsT (stationary) = wT tile   [128k, 128o]   fp16 (ternary, exact)
  rhs  (moving)     = xT tile   [128k, 512t]   fp16
  psum              = outT tile [128o, 512t]   fp32, accumulated over 32 k-tiles
"""

import os
import numpy as np

N_CORES = 8
T = 8192      # tokens (rows of x)
K = 4096      # in_features (contraction)
O = 16384     # out_features
O_C = O // N_CORES   # 2048 per core
P = 128
TN = 512             # moving free dim / PSUM bank width (fp32)
KT = K // P          # 32 k-tiles
TC = T // TN         # 16 token chunks
OT = O_C // P        # 16 out-feature tiles per core

EPS = 1e-6
THRESHOLD = 0.5

# Filled by the last kernel() call when tracing is enabled (BITLIN_TRACE=1).
LAST_EXEC_TIME_NS = None
LAST_RESULTS = None

_PROGRAM_CACHE = {}


def _install_trace_shim():
    """Make run_bass_kernel_spmd(trace=True) work in images whose antenv
    package lacks axon_hooks. Dev-only path (BITLIN_TRACE=1)."""
    import sys, types
    if "antenv.axon_hooks" not in sys.modules:
        import antenv
        hooks = types.ModuleType("antenv.axon_hooks")
        _store = {"h": None}
        hooks.set_axon_ntff_profile_hook = lambda h: _store.__setitem__("h", h)
        hooks.get_axon_ntff_profile_hook = lambda: _store["h"]
        sys.modules["antenv.axon_hooks"] = hooks
        antenv.axon_hooks = hooks
    from antenv.axon_hooks import (
        get_axon_ntff_profile_hook,
        set_axon_ntff_profile_hook,
    )
    if get_axon_ntff_profile_hook() is None:
        from trn_agent_boot.trn_boot import _ntff_profile_via_ctypes
        set_axon_ntff_profile_hook(
            _ntff_profile_via_ctypes("/opt/axon/libaxon_pjrt.so")
        )
    import concourse.bass_utils as bu
    bu.upload_artifacts = lambda tmpdir: f"local:{tmpdir}"


def _build_program():
    import concourse.bacc as bacc
    import concourse.mybir as mybir
    from concourse.tile import TileContext

    f16 = mybir.dt.float16
    f32 = mybir.dt.float32
    Identity = mybir.ActivationFunctionType.Identity

    nc = bacc.Bacc(
        "TRN2", target_bir_lowering=False, debug=False, num_devices=N_CORES
    )
    xt = nc.dram_tensor("xt", [K, T], f16, kind="ExternalInput")
    wt = nc.dram_tensor("wt", [K, O_C], f16, kind="ExternalInput")
    bias = nc.dram_tensor("bias", [P, OT], f32, kind="ExternalInput")
    scl = nc.dram_tensor("scl", [P, 1], f32, kind="ExternalInput")
    outt = nc.dram_tensor("outt", [O_C, T], f32, kind="ExternalOutput")

    with TileContext(nc) as tc:
        with (
            tc.tile_pool(name="wpool", bufs=KT) as wpool,
            tc.tile_pool(name="xpool", bufs=40) as xpool,
            tc.tile_pool(name="cpool", bufs=1) as cpool,
            tc.tile_pool(name="opool", bufs=4) as opool,
            tc.tile_pool(name="pspool", bufs=4, space="PSUM") as pspool,
        ):
            bias_t = cpool.tile([P, OT], f32, tag="bias")
            nc.sync.dma_start(out=bias_t[:], in_=bias.ap()[:, :])
            scl_t = cpool.tile([P, 1], f32, tag="scl")
            nc.sync.dma_start(out=scl_t[:], in_=scl.ap()[:, :])

            # Whole weight slice stays resident in SBUF (32 x 4KB/partition).
            wtiles = []
            for k in range(KT):
                w_tile = wpool.tile([P, O_C], f16, tag="w")
                nc.sync.dma_start(
                    out=w_tile[:], in_=wt.ap()[k * P : (k + 1) * P, :]
                )
                wtiles.append(w_tile)

            for tci in range(TC):
                xtiles = []
                for k in range(KT):
                    x_tile = xpool.tile([P, TN], f16, tag="x")
                    nc.sync.dma_start(
                        out=x_tile[:],
                        in_=xt.ap()[k * P : (k + 1) * P, tci * TN : (tci + 1) * TN],
                    )
                    xtiles.append(x_tile)
                for o in range(OT):
                    ps = pspool.tile([P, TN], f32, tag="ps")
                    for k in range(KT):
                        nc.tensor.matmul(
                            ps[:],
                            wtiles[k][:, o * P : (o + 1) * P],
                            xtiles[k][:],
                            start=(k == 0),
                            stop=(k == KT - 1),
                        )
                    o_tile = opool.tile([P, TN], f32, tag="o")
                    nc.scalar.activation(
                        o_tile[:],
                        ps[:],
                        Identity,
                        bias=bias_t[:, o : o + 1],
                        scale=scl_t[:, 0:1],
                    )
                    nc.sync.dma_start(
                        out=outt.ap()[
                            o * P : (o + 1) * P, tci * TN : (tci + 1) * TN
                        ],
                        in_=o_tile[:],
                    )

    nc.compile()
    return nc


def kernel(x: np.ndarray, weight: np.ndarray, bias: np.ndarray) -> np.ndarray:
    global LAST_EXEC_TIME_NS, LAST_RESULTS
    from concourse.bass_utils import run_bass_kernel_spmd

    trace = os.environ.get("BITLIN_TRACE", "") == "1"
    if trace:
        _install_trace_shim()

    x = np.asarray(x, dtype=np.float32)
    weight = np.asarray(weight, dtype=np.float32)
    bias = np.asarray(bias, dtype=np.float32)

    # --- host-side quantization (cheap; the matmul is the device's job) ---
    scale = np.float32(max(np.abs(weight).mean(dtype=np.float64), EPS))
    xt16 = x.T.astype(np.float16)                       # (K, T)
    scl_arr = np.full((P, 1), scale, dtype=np.float32)

    in_maps = []
    for c in range(N_CORES):
        w_c = weight[c * O_C : (c + 1) * O_C]           # (O_C, K) f32
        normalized = w_c / scale
        tern = np.sign(normalized, dtype=np.float32)
        tern *= (np.abs(normalized) > THRESHOLD).astype(np.float32)
        wt_c = tern.T.astype(np.float16)                # (K, O_C), {-1,0,1} exact
        bias_c = np.ascontiguousarray(
            bias[c * O_C : (c + 1) * O_C].reshape(OT, P).T
        )                                               # (P, OT): [p, j] = b[j*128+p]
        in_maps.append(
            {"xt": xt16, "wt": wt_c, "bias": bias_c, "scl": scl_arr}
        )

    if "prog" not in _PROGRAM_CACHE:
        _PROGRAM_CACHE["prog"] = _build_program()
    nc = _PROGRAM_CACHE["prog"]

    kwargs = {}
    if trace:
        kwargs = {"trace": True, "tmpdir": os.environ.get("BITLIN_TRACE_DIR")}
    res = run_bass_kernel_spmd(nc, in_maps, list(range(N_CORES)), **kwargs)
    LAST_EXEC_TIME_NS = res.exec_time_ns
    LAST_RESULTS = res

    out = np.empty((T, O), dtype=np.float32)
    for c in range(N_CORES):
        out[:, c * O_C : (c + 1) * O_C] = res.results[c]["outt"].T
    return out


# revision 3
# speedup vs baseline: 1.0604x; 1.0604x over previous
"""BitLinear (ternary-quantized linear) Trainium2 kernel, 8-way tensor-parallel.

Computes  out = x @ quantize(weight).T + bias  for
  x      (8192, 4096) f32
  weight (16384, 4096) f32
  bias   (16384,) f32
  out    (8192, 16384) f32

quantize(w) = ternarize(w / scale) * scale with scale = max(mean|w|, 1e-6),
ternary in {-1, 0, +1}.

Strategy (column-parallel linear per the tensor-parallel sharding):
  - Host: compute scale, ternarize weights (exactly representable in fp16),
    cast x to fp16, pre-transpose both so the device does no transposes.
  - Each of the 8 cores holds a 2048-wide slice of out_features, streams the
    full x once, and computes outT_c = (wT_c.T @ xT) with fp32 PSUM
    accumulation; the ACT engine applies  *scale + bias  on PSUM eviction.
  - No collectives: the host concatenates the 8 column slices.

Device layout per core (out^T orientation — out_features on partitions):
  lhsT (stationary) = wT tile   [128k, 128o]   fp16 (ternary, exact)
  rhs  (moving)     = xT tile   [128k, 512t]   fp16
  psum              = outT tile [128o, 512t]   fp32, accumulated over 32 k-tiles
"""

import os
import numpy as np

N_CORES = 8
T = 8192      # tokens (rows of x)
K = 4096      # in_features (contraction)
O = 16384     # out_features
O_C = O // N_CORES   # 2048 per core
P = 128
TN = 512             # moving free dim / PSUM bank width (fp32)
KT = K // P          # 32 k-tiles
TC = T // TN         # 16 token chunks
OT = O_C // P        # 16 out-feature tiles per core

EPS = 1e-6
THRESHOLD = 0.5

# Filled by the last kernel() call when tracing is enabled (BITLIN_TRACE=1).
LAST_EXEC_TIME_NS = None
LAST_RESULTS = None

_PROGRAM_CACHE = {}


def _install_trace_shim():
    """Make run_bass_kernel_spmd(trace=True) work in images whose antenv
    package lacks axon_hooks. Dev-only path (BITLIN_TRACE=1)."""
    import sys, types
    if "antenv.axon_hooks" not in sys.modules:
        import antenv
        hooks = types.ModuleType("antenv.axon_hooks")
        _store = {"h": None}
        hooks.set_axon_ntff_profile_hook = lambda h: _store.__setitem__("h", h)
        hooks.get_axon_ntff_profile_hook = lambda: _store["h"]
        sys.modules["antenv.axon_hooks"] = hooks
        antenv.axon_hooks = hooks
    from antenv.axon_hooks import (
        get_axon_ntff_profile_hook,
        set_axon_ntff_profile_hook,
    )
    if get_axon_ntff_profile_hook() is None:
        from trn_agent_boot.trn_boot import _ntff_profile_via_ctypes
        set_axon_ntff_profile_hook(
            _ntff_profile_via_ctypes("/opt/axon/libaxon_pjrt.so")
        )
    import concourse.bass_utils as bu
    bu.upload_artifacts = lambda tmpdir: f"local:{tmpdir}"


def _build_program():
    import concourse.bacc as bacc
    import concourse.mybir as mybir
    from concourse.tile import TileContext

    f16 = mybir.dt.float16
    f32 = mybir.dt.float32
    Identity = mybir.ActivationFunctionType.Identity

    nc = bacc.Bacc(
        "TRN2", target_bir_lowering=False, debug=False, num_devices=N_CORES
    )
    xt = nc.dram_tensor("xt", [K, T], f16, kind="ExternalInput")
    wt = nc.dram_tensor("wt", [K, O_C], f16, kind="ExternalInput")
    bias = nc.dram_tensor("bias", [P, OT], f32, kind="ExternalInput")
    scl = nc.dram_tensor("scl", [P, 1], f32, kind="ExternalInput")
    outt = nc.dram_tensor("outt", [O_C, T], f32, kind="ExternalOutput")

    OB = 4              # o-tiles per block (PSUM banks per block; 2 blocks in flight)
    NB = OT // OB       # 4 o-blocks
    WS = O_C // TN      # 4 weight column-slices per k-tile

    with TileContext(nc) as tc:
        with (
            tc.tile_pool(name="wpool", bufs=KT * WS) as wpool,
            tc.tile_pool(name="xpool", bufs=40) as xpool,
            tc.tile_pool(name="cpool", bufs=1) as cpool,
            tc.tile_pool(name="opool", bufs=4) as opool,
            tc.tile_pool(name="pspool", bufs=8, space="PSUM") as pspool,
        ):
            bias_t = cpool.tile([P, OT], f32, tag="bias")
            nc.sync.dma_start(out=bias_t[:], in_=bias.ap()[:, :])
            scl_t = cpool.tile([P, 1], f32, tag="scl")
            nc.sync.dma_start(out=scl_t[:], in_=scl.ap()[:, :])

            def x_dma(tci, k):
                x_tile = xpool.tile([P, TN], f16, tag="x")
                nc.sync.dma_start(
                    out=x_tile[:],
                    in_=xt.ap()[k * P : (k + 1) * P, tci * TN : (tci + 1) * TN],
                )
                return x_tile

            # Weights stay fully SBUF-resident as (KT x WS) tiles of [128, 512].
            # Emission order matters: x(tc0) and the first weight column-slice
            # are interleaved k-by-k so the very first o-block is compute-paced
            # rather than blocked on the whole 16MB weight load.
            wtiles = [[None] * WS for _ in range(KT)]

            def w_dma(k, s):
                w_tile = wpool.tile([P, TN], f16, tag="w")
                nc.sync.dma_start(
                    out=w_tile[:],
                    in_=wt.ap()[k * P : (k + 1) * P, s * TN : (s + 1) * TN],
                )
                wtiles[k][s] = w_tile

            xtiles0 = []
            for k in range(KT):
                xtiles0.append(x_dma(0, k))
                w_dma(k, 0)
            for s in range(1, WS):
                for k in range(KT):
                    w_dma(k, s)

            for tci in range(TC):
                xtiles = xtiles0 if tci == 0 else [x_dma(tci, k) for k in range(KT)]
                for ob in range(NB):
                    pss = [pspool.tile([P, TN], f32, tag="ps", name="ps") for _ in range(OB)]
                    for k in range(KT):
                        for oi in range(OB):
                            o = ob * OB + oi
                            nc.tensor.matmul(
                                pss[oi][:],
                                wtiles[k][o // OB][:, (o % OB) * P : (o % OB + 1) * P],
                                xtiles[k][:],
                                start=(k == 0),
                                stop=(k == KT - 1),
                            )
                    for oi in range(OB):
                        o = ob * OB + oi
                        o_tile = opool.tile([P, TN], f32, tag="o")
                        nc.scalar.activation(
                            o_tile[:],
                            pss[oi][:],
                            Identity,
                            bias=bias_t[:, o : o + 1],
                            scale=scl_t[:, 0:1],
                        )
                        nc.sync.dma_start(
                            out=outt.ap()[
                                o * P : (o + 1) * P, tci * TN : (tci + 1) * TN
                            ],
                            in_=o_tile[:],
                        )

    nc.compile()
    return nc


def kernel(x: np.ndarray, weight: np.ndarray, bias: np.ndarray) -> np.ndarray:
    global LAST_EXEC_TIME_NS, LAST_RESULTS
    from concourse.bass_utils import run_bass_kernel_spmd

    trace = os.environ.get("BITLIN_TRACE", "") == "1"
    if trace:
        _install_trace_shim()

    x = np.asarray(x, dtype=np.float32)
    weight = np.asarray(weight, dtype=np.float32)
    bias = np.asarray(bias, dtype=np.float32)

    # --- host-side quantization (cheap; the matmul is the device's job) ---
    scale = np.float32(max(np.abs(weight).mean(dtype=np.float64), EPS))
    xt16 = x.T.astype(np.float16)                       # (K, T)
    scl_arr = np.full((P, 1), scale, dtype=np.float32)

    in_maps = []
    for c in range(N_CORES):
        w_c = weight[c * O_C : (c + 1) * O_C]           # (O_C, K) f32
        normalized = w_c / scale
        tern = np.sign(normalized, dtype=np.float32)
        tern *= (np.abs(normalized) > THRESHOLD).astype(np.float32)
        wt_c = tern.T.astype(np.float16)                # (K, O_C), {-1,0,1} exact
        bias_c = np.ascontiguousarray(
            bias[c * O_C : (c + 1) * O_C].reshape(OT, P).T
        )                                               # (P, OT): [p, j] = b[j*128+p]
        in_maps.append(
            {"xt": xt16, "wt": wt_c, "bias": bias_c, "scl": scl_arr}
        )

    if "prog" not in _PROGRAM_CACHE:
        _PROGRAM_CACHE["prog"] = _build_program()
    nc = _PROGRAM_CACHE["prog"]

    kwargs = {}
    if trace:
        kwargs = {"trace": True, "tmpdir": os.environ.get("BITLIN_TRACE_DIR")}
    res = run_bass_kernel_spmd(nc, in_maps, list(range(N_CORES)), **kwargs)
    LAST_EXEC_TIME_NS = res.exec_time_ns
    LAST_RESULTS = res

    out = np.empty((T, O), dtype=np.float32)
    for c in range(N_CORES):
        out[:, c * O_C : (c + 1) * O_C] = res.results[c]["outt"].T
    return out
